# revision 8
# baseline (speedup 1.0000x reference)
"""Trainium2 Bass kernel for nn_KalmanGraphicalModel (gnn_message_passing).

The reference runs ITERS=100 iterations of a LINEAR 3-point stencil in time:
    x <- A' x_t + B' x_{t-1} + C' x_{t+1} + Gam y_t     (edge-replicated)
Because the update is linear and gamma is small, the composed 100-step
operator is a banded convolution with numerically tiny bandwidth D (~12 for
gamma=0.01):
    x_100[t] = sum_{|d|<=D} G_d x0[t+d] + V_d y[t+d]
So the whole problem collapses to ONE banded-matmul pass on device:
  - time axis folded 16-way into the partition dim (16 blocks x 8 rows = 128)
  - the stencil taps become 128x128 block-banded weight matrices; taps that
    cross a fold boundary land in neighbor-column streams (sigma = -S..S)
  - per 512-col tile: (2S+1) x-matmuls + (2S+1) y-matmuls accumulate in PSUM
T is sharded across 8 cores; the first/last 128 columns (edge-rule
influenced + window zero-padding) are computed host-side on tiny strips.
"""
import os
import numpy as np

N, M, T, ITERS = 8, 4, 500000, 100
NCORES = 8
L = T // NCORES          # 62500 timesteps per core
FOLD = 16                # time-fold factor -> 16 blocks x 8 rows = 128 partitions
NC = 3908                # out cols per core: 16*3908 = 62528 >= 62500
EDGE = 128               # host-computed override width at the two true edges
STRIP = 384              # width of host edge strips
TAU = 1e-10              # tap truncation threshold (relative)

_PROGRAM_CACHE = {}


def _compose_taps(F, H, Q, R, gamma):
    """Banded composition of the 100 linear steps, in float64."""
    Qinv = np.linalg.inv(Q)
    Rinv = np.linalg.inv(R)
    negQinv = -Qinv
    FtQinv = F.T @ Qinv
    HtRinv = H.T @ Rinv
    Z1 = np.eye(N); Z1[0, 0] = 0.0
    Z2 = np.eye(N); Z2[-1, -1] = 0.0
    Ap = np.eye(N) + gamma * (negQinv @ Z1 - FtQinv @ Z2 @ F - HtRinv @ H)
    Bp = -gamma * (negQinv @ Z1 @ F)
    Cp = gamma * (FtQinv @ Z2)
    Gam = gamma * HtRinv

    K = ITERS
    G = np.zeros((2 * K + 1, N, N))
    V = np.zeros((2 * K + 1, N, M))
    G[K] = np.eye(N)
    for _ in range(K):
        Gn = np.einsum("ij,djk->dik", Ap, G)
        Gn[:-1] += np.einsum("ij,djk->dik", Bp, G[1:])
        Gn[1:] += np.einsum("ij,djk->dik", Cp, G[:-1])
        Vn = np.einsum("ij,djk->dik", Ap, V)
        Vn[:-1] += np.einsum("ij,djk->dik", Bp, V[1:])
        Vn[1:] += np.einsum("ij,djk->dik", Cp, V[:-1])
        Vn[K] += Gam
        G, V = Gn, Vn

    gmax = np.abs(G).max(axis=(1, 2))
    vmax = np.abs(V).max(axis=(1, 2))
    scale = max(gmax.max(), vmax.max())
    keep = np.where((gmax > TAU * scale) | (vmax > TAU * scale))[0]
    D = int(max(1, np.abs(keep - K).max()))
    return G, V, D, (Ap.astype(np.float32), Bp.astype(np.float32),
                     Cp.astype(np.float32), Gam.astype(np.float32))


def _build_program(S):
    """Build + schedule the Bass/Tile program (cached per S)."""
    import concourse.bass as bass
    import concourse.tile as tile
    from concourse import bacc, mybir

    if S in _PROGRAM_CACHE:
        return _PROGRAM_CACHE[S]

    CW = NC + 2 * S
    nsig = 2 * S + 1
    f32 = mybir.dt.float32
    f32r = mybir.dt.float32r

    nc = bacc.Bacc("TRN2", target_bir_lowering=False, debug=False,
                   enable_asserts=False, num_devices=NCORES)
    xf = nc.dram_tensor("xf", [128, CW], f32r, kind="ExternalInput").ap()
    yf = nc.dram_tensor("yf", [64, CW], f32r, kind="ExternalInput").ap()
    wx = nc.dram_tensor("wx", [128, nsig * 128], f32r, kind="ExternalInput").ap()
    wy = nc.dram_tensor("wy", [64, nsig * 128], f32r, kind="ExternalInput").ap()
    out = nc.dram_tensor("out", [128, NC], f32, kind="ExternalOutput").ap()

    TS = 512
    tiles = []
    c = 0
    while c < NC:
        tiles.append((c, min(TS, NC - c)))
        c += TS

    with tile.TileContext(nc) as tc:
        with tc.tile_pool(name="consts", bufs=1) as consts, \
             tc.tile_pool(name="ps", bufs=4, space="PSUM") as ps_pool, \
             tc.tile_pool(name="outp", bufs=4) as outp:
            wxsb = consts.tile([128, nsig * 128], f32r)
            wysb = consts.tile([64, nsig * 128], f32r)
            nc.sync.dma_start(wxsb[:], wx[:])
            nc.sync.dma_start(wysb[:], wy[:])
            xsb = consts.tile([128, CW], f32r)
            ysb = consts.tile([64, CW], f32r)
            # chunked loads so compute can start before the whole window lands
            for (c0, cn) in tiles:
                nc.sync.dma_start(xsb[:, c0:c0 + cn], xf[:, c0:c0 + cn])
                nc.sync.dma_start(ysb[:, c0:c0 + cn], yf[:, c0:c0 + cn])
            nc.sync.dma_start(xsb[:, NC:CW], xf[:, NC:CW])
            nc.sync.dma_start(ysb[:, NC:CW], yf[:, NC:CW])
            for (c0, cn) in tiles:
                ps = ps_pool.tile([128, cn], f32)
                for si in range(nsig):
                    nc.tensor.matmul(
                        ps[:],
                        wxsb[:, si * 128:(si + 1) * 128],
                        xsb[:, c0 + si:c0 + si + cn],
                        start=(si == 0), stop=False)
                for si in range(nsig):
                    nc.tensor.matmul(
                        ps[:],
                        wysb[:, si * 128:(si + 1) * 128],
                        ysb[:, c0 + si:c0 + si + cn],
                        start=False, stop=(si == nsig - 1))
                ot = outp.tile([128, cn], f32)
                nc.vector.tensor_copy(ot[:], ps[:])
                nc.sync.dma_start(out[:, c0:c0 + cn], ot[:])
    nc.compile()
    _PROGRAM_CACHE[S] = nc
    return nc


def _fold(a, rows, CW):
    # a: (rows, 16*CW) -> (rows*16 partitions, CW); partition b*rows+r holds
    # times t = c*16+b
    return np.ascontiguousarray(
        a.reshape(rows, CW, FOLD).transpose(2, 0, 1).reshape(FOLD * rows, CW))


def _run_edge_strip(x0, y, Ap, Bp, Cp, Gam):
    # reference-style edge replication on both strip ends; only the true-edge
    # side of the strip is consumed, the other side's garbage stays >100 cols
    # away from the EDGE-wide region we keep.
    x = x0.copy()
    for _ in range(ITERS):
        xp = np.concatenate([x[:, :1], x[:, :-1]], axis=1)
        xf_ = np.concatenate([x[:, 1:], x[:, -1:]], axis=1)
        x = (Ap @ x + Bp @ xp + Cp @ xf_ + Gam @ y).astype(np.float32)
    return x


def kernel(xs, ys, F, H, Q, R, gamma):
    from concourse.bass_utils import run_bass_kernel_spmd

    xs = np.asarray(xs, dtype=np.float32)
    ysv = np.asarray(ys, dtype=np.float32)
    F64 = np.asarray(F, dtype=np.float64)
    H64 = np.asarray(H, dtype=np.float64)
    Q64 = np.asarray(Q, dtype=np.float64)
    R64 = np.asarray(R, dtype=np.float64)
    g = float(np.asarray(gamma))

    G, V, D, mats32 = _compose_taps(F64, H64, Q64, R64, g)
    S = (D + FOLD - 1) // FOLD
    assert S <= 7, f"bandwidth D={D} too large for single-pass kernel"
    CW = NC + 2 * S
    nsig = 2 * S + 1

    # ---- weights ----
    K = ITERS
    WX = np.zeros((nsig, 128, 128), dtype=np.float32)
    WY = np.zeros((nsig, 64, 128), dtype=np.float32)
    for si in range(nsig):
        sig = si - S
        for bo in range(FOLD):
            for bi in range(FOLD):
                d = sig * FOLD + bi - bo
                if abs(d) > D:
                    continue
                WX[si, bi * 8:bi * 8 + 8, bo * 8:bo * 8 + 8] = G[K + d].T
                WY[si, bi * 4:bi * 4 + 4, bo * 8:bo * 8 + 8] = V[K + d].T

    # ---- per-core folded input windows ----
    pad = FOLD * S
    padR = pad + (FOLD * NC - L)          # right overhang of core 7's window
    xs_p = np.zeros((N, T + pad + padR), dtype=np.float32)
    ys_p = np.zeros((M, T + pad + padR), dtype=np.float32)
    xs_p[:, pad:pad + T] = xs
    ys_p[:, pad:pad + T] = ysv
    # SBUF weight tile is (parts, nsig*128), sigma-major along columns
    wx_np = np.ascontiguousarray(WX.transpose(1, 0, 2).reshape(128, nsig * 128))
    wy_np = np.ascontiguousarray(WY.transpose(1, 0, 2).reshape(64, nsig * 128))
    in_maps = []
    for i in range(NCORES):
        o = i * L
        in_maps.append({
            "xf": _fold(xs_p[:, o:o + FOLD * CW], N, CW),
            "yf": _fold(ys_p[:, o:o + FOLD * CW], M, CW),
            "wx": wx_np,
            "wy": wy_np,
        })

    nc = _build_program(S)
    trace = bool(int(os.environ.get("KALMAN_TRACE", "0")))
    res = run_bass_kernel_spmd(nc, in_maps, core_ids=list(range(NCORES)),
                               trace=trace)
    if trace and res.exec_time_ns is not None:
        print(f"HW exec time: {res.exec_time_ns} ns")
        print(f"HW exec time mean: {res.mean_exec_time_ns} ns")

    out_full = np.empty((N, T), dtype=np.float32)
    for i in range(NCORES):
        o = i * L
        Out = res.results[i]["out"]                       # (128, NC)
        unf = Out.reshape(FOLD, N, NC).transpose(1, 2, 0).reshape(N, FOLD * NC)
        out_full[:, o:o + L] = unf[:, :L]

    # ---- host edge strips (exact edge-replication dynamics) ----
    Ap32, Bp32, Cp32, Gam32 = mats32
    left = _run_edge_strip(xs[:, :STRIP], ysv[:, :STRIP],
                           Ap32, Bp32, Cp32, Gam32)
    right = _run_edge_strip(xs[:, -STRIP:], ysv[:, -STRIP:],
                            Ap32, Bp32, Cp32, Gam32)
    out_full[:, :EDGE] = left[:, :EDGE]
    out_full[:, -EDGE:] = right[:, -EDGE:]
    return out_full


# revision 9
# speedup vs baseline: 1.0447x; 1.0447x over previous
"""Trainium2 Bass kernel for nn_KalmanGraphicalModel (gnn_message_passing).

The reference runs ITERS=100 iterations of a LINEAR 3-point stencil in time:
    x <- A' x_t + B' x_{t-1} + C' x_{t+1} + Gam y_t     (edge-replicated)
Because the update is linear and gamma is small, the composed 100-step
operator is a banded convolution with numerically tiny bandwidth D (~12 for
gamma=0.01):
    x_100[t] = sum_{|d|<=D} G_d x0[t+d] + V_d y[t+d]
So the whole problem collapses to ONE banded-matmul pass on device:
  - time axis folded 16-way into the partition dim (16 blocks x 8 rows = 128)
  - the stencil taps become 128x128 block-banded weight matrices; taps that
    cross a fold boundary land in neighbor-column streams (sigma = -S..S)
  - per 512-col tile: (2S+1) x-matmuls + (2S+1) y-matmuls accumulate in PSUM
T is sharded across 8 cores; the first/last 128 columns (edge-rule
influenced + window zero-padding) are computed host-side on tiny strips.
"""
import os
import numpy as np

N, M, T, ITERS = 8, 4, 500000, 100
NCORES = 8
L = T // NCORES          # 62500 timesteps per core
FOLD = 16                # time-fold factor -> 16 blocks x 8 rows = 128 partitions
NC = 3908                # out cols per core: 16*3908 = 62528 >= 62500
EDGE = 128               # host-computed override width at the two true edges
STRIP = 384              # width of host edge strips
TAU = 1e-10              # tap truncation threshold (relative)

_PROGRAM_CACHE = {}


def _compose_taps(F, H, Q, R, gamma):
    """Banded composition of the 100 linear steps, in float64."""
    Qinv = np.linalg.inv(Q)
    Rinv = np.linalg.inv(R)
    negQinv = -Qinv
    FtQinv = F.T @ Qinv
    HtRinv = H.T @ Rinv
    Z1 = np.eye(N); Z1[0, 0] = 0.0
    Z2 = np.eye(N); Z2[-1, -1] = 0.0
    Ap = np.eye(N) + gamma * (negQinv @ Z1 - FtQinv @ Z2 @ F - HtRinv @ H)
    Bp = -gamma * (negQinv @ Z1 @ F)
    Cp = gamma * (FtQinv @ Z2)
    Gam = gamma * HtRinv

    K = ITERS
    G = np.zeros((2 * K + 1, N, N))
    V = np.zeros((2 * K + 1, N, M))
    G[K] = np.eye(N)
    for _ in range(K):
        Gn = np.einsum("ij,djk->dik", Ap, G)
        Gn[:-1] += np.einsum("ij,djk->dik", Bp, G[1:])
        Gn[1:] += np.einsum("ij,djk->dik", Cp, G[:-1])
        Vn = np.einsum("ij,djk->dik", Ap, V)
        Vn[:-1] += np.einsum("ij,djk->dik", Bp, V[1:])
        Vn[1:] += np.einsum("ij,djk->dik", Cp, V[:-1])
        Vn[K] += Gam
        G, V = Gn, Vn

    gmax = np.abs(G).max(axis=(1, 2))
    vmax = np.abs(V).max(axis=(1, 2))
    scale = max(gmax.max(), vmax.max())
    keep = np.where((gmax > TAU * scale) | (vmax > TAU * scale))[0]
    D = int(max(1, np.abs(keep - K).max()))
    return G, V, D, (Ap.astype(np.float32), Bp.astype(np.float32),
                     Cp.astype(np.float32), Gam.astype(np.float32))


def _build_program(S):
    """Build + schedule the Bass/Tile program (cached per S)."""
    import concourse.bass as bass
    import concourse.tile as tile
    from concourse import bacc, mybir

    if S in _PROGRAM_CACHE:
        return _PROGRAM_CACHE[S]

    CW = NC + 2 * S
    nsig = 2 * S + 1
    f32 = mybir.dt.float32
    f32r = mybir.dt.float32r

    nc = bacc.Bacc("TRN2", target_bir_lowering=False, debug=False,
                   enable_asserts=False, num_devices=NCORES)
    xf = nc.dram_tensor("xf", [128, CW], f32r, kind="ExternalInput").ap()
    yf = nc.dram_tensor("yf", [64, CW], f32r, kind="ExternalInput").ap()
    wx = nc.dram_tensor("wx", [128, nsig * 128], f32r, kind="ExternalInput").ap()
    wy = nc.dram_tensor("wy", [64, nsig * 128], f32r, kind="ExternalInput").ap()
    out = nc.dram_tensor("out", [128, NC], f32, kind="ExternalOutput").ap()

    TS = 512
    tiles = []
    c = 0
    while c < NC:
        tiles.append((c, min(TS, NC - c)))
        c += TS

    with tile.TileContext(nc) as tc:
        with tc.tile_pool(name="consts", bufs=1) as consts, \
             tc.tile_pool(name="ps", bufs=8, space="PSUM") as ps_pool, \
             tc.tile_pool(name="outp", bufs=8) as outp:
            wxsb = consts.tile([128, nsig * 128], f32r)
            wysb = consts.tile([64, nsig * 128], f32r)
            # scalar (Activation HWDGE) carries weights + y; sync carries x.
            # The two direct-DMA streams run in parallel, so the first
            # accumulation group's operands land ~2us in instead of ~13us.
            nc.scalar.dma_start(wysb[:], wy[:])
            nc.scalar.dma_start(wxsb[:], wx[:])
            xsb = consts.tile([128, CW], f32r)
            ysb = consts.tile([64, CW], f32r)
            # chunked loads so compute can start before the whole window lands
            for (c0, cn) in tiles:
                nc.sync.dma_start(xsb[:, c0:c0 + cn], xf[:, c0:c0 + cn])
                nc.scalar.dma_start(ysb[:, c0:c0 + cn], yf[:, c0:c0 + cn])
            nc.sync.dma_start(xsb[:, NC:CW], xf[:, NC:CW])
            nc.scalar.dma_start(ysb[:, NC:CW], yf[:, NC:CW])
            for ti, (c0, cn) in enumerate(tiles):
                ps = ps_pool.tile([128, cn], f32)
                for si in range(nsig):
                    nc.tensor.matmul(
                        ps[:],
                        wysb[:, si * 128:(si + 1) * 128],
                        ysb[:, c0 + si:c0 + si + cn],
                        start=(si == 0), stop=False)
                for si in range(nsig):
                    nc.tensor.matmul(
                        ps[:],
                        wxsb[:, si * 128:(si + 1) * 128],
                        xsb[:, c0 + si:c0 + si + cn],
                        start=False, stop=(si == nsig - 1))
                ot = outp.tile([128, cn], f32)
                nc.vector.tensor_copy(ot[:], ps[:])
                eng = nc.scalar if ti % 2 else nc.sync
                eng.dma_start(out[:, c0:c0 + cn], ot[:])
    nc.compile()
    _PROGRAM_CACHE[S] = nc
    return nc


def _fold(a, rows, CW):
    # a: (rows, 16*CW) -> (rows*16 partitions, CW); partition b*rows+r holds
    # times t = c*16+b
    return np.ascontiguousarray(
        a.reshape(rows, CW, FOLD).transpose(2, 0, 1).reshape(FOLD * rows, CW))


def _run_edge_strip(x0, y, Ap, Bp, Cp, Gam):
    # reference-style edge replication on both strip ends; only the true-edge
    # side of the strip is consumed, the other side's garbage stays >100 cols
    # away from the EDGE-wide region we keep.
    x = x0.copy()
    for _ in range(ITERS):
        xp = np.concatenate([x[:, :1], x[:, :-1]], axis=1)
        xf_ = np.concatenate([x[:, 1:], x[:, -1:]], axis=1)
        x = (Ap @ x + Bp @ xp + Cp @ xf_ + Gam @ y).astype(np.float32)
    return x


def kernel(xs, ys, F, H, Q, R, gamma):
    from concourse.bass_utils import run_bass_kernel_spmd

    xs = np.asarray(xs, dtype=np.float32)
    ysv = np.asarray(ys, dtype=np.float32)
    F64 = np.asarray(F, dtype=np.float64)
    H64 = np.asarray(H, dtype=np.float64)
    Q64 = np.asarray(Q, dtype=np.float64)
    R64 = np.asarray(R, dtype=np.float64)
    g = float(np.asarray(gamma))

    G, V, D, mats32 = _compose_taps(F64, H64, Q64, R64, g)
    S = (D + FOLD - 1) // FOLD
    assert S <= 7, f"bandwidth D={D} too large for single-pass kernel"
    CW = NC + 2 * S
    nsig = 2 * S + 1

    # ---- weights ----
    K = ITERS
    WX = np.zeros((nsig, 128, 128), dtype=np.float32)
    WY = np.zeros((nsig, 64, 128), dtype=np.float32)
    for si in range(nsig):
        sig = si - S
        for bo in range(FOLD):
            for bi in range(FOLD):
                d = sig * FOLD + bi - bo
                if abs(d) > D:
                    continue
                WX[si, bi * 8:bi * 8 + 8, bo * 8:bo * 8 + 8] = G[K + d].T
                WY[si, bi * 4:bi * 4 + 4, bo * 8:bo * 8 + 8] = V[K + d].T

    # ---- per-core folded input windows ----
    pad = FOLD * S
    padR = pad + (FOLD * NC - L)          # right overhang of core 7's window
    xs_p = np.zeros((N, T + pad + padR), dtype=np.float32)
    ys_p = np.zeros((M, T + pad + padR), dtype=np.float32)
    xs_p[:, pad:pad + T] = xs
    ys_p[:, pad:pad + T] = ysv
    # SBUF weight tile is (parts, nsig*128), sigma-major along columns
    wx_np = np.ascontiguousarray(WX.transpose(1, 0, 2).reshape(128, nsig * 128))
    wy_np = np.ascontiguousarray(WY.transpose(1, 0, 2).reshape(64, nsig * 128))
    in_maps = []
    for i in range(NCORES):
        o = i * L
        in_maps.append({
            "xf": _fold(xs_p[:, o:o + FOLD * CW], N, CW),
            "yf": _fold(ys_p[:, o:o + FOLD * CW], M, CW),
            "wx": wx_np,
            "wy": wy_np,
        })

    nc = _build_program(S)
    trace = bool(int(os.environ.get("KALMAN_TRACE", "0")))
    res = run_bass_kernel_spmd(nc, in_maps, core_ids=list(range(NCORES)),
                               trace=trace)
    if trace and res.exec_time_ns is not None:
        print(f"HW exec time: {res.exec_time_ns} ns")
        print(f"HW exec time mean: {res.mean_exec_time_ns} ns")

    out_full = np.empty((N, T), dtype=np.float32)
    for i in range(NCORES):
        o = i * L
        Out = res.results[i]["out"]                       # (128, NC)
        unf = Out.reshape(FOLD, N, NC).transpose(1, 2, 0).reshape(N, FOLD * NC)
        out_full[:, o:o + L] = unf[:, :L]

    # ---- host edge strips (exact edge-replication dynamics) ----
    Ap32, Bp32, Cp32, Gam32 = mats32
    left = _run_edge_strip(xs[:, :STRIP], ysv[:, :STRIP],
                           Ap32, Bp32, Cp32, Gam32)
    right = _run_edge_strip(xs[:, -STRIP:], ysv[:, -STRIP:],
                            Ap32, Bp32, Cp32, Gam32)
    out_full[:, :EDGE] = left[:, :EDGE]
    out_full[:, -EDGE:] = right[:, -EDGE:]
    return out_full


# revision 10
# speedup vs baseline: 1.1188x; 1.0709x over previous
"""Trainium2 Bass kernel for nn_KalmanGraphicalModel (gnn_message_passing).

The reference runs ITERS=100 iterations of a LINEAR 3-point stencil in time:
    x <- A' x_t + B' x_{t-1} + C' x_{t+1} + Gam y_t     (edge-replicated)
Because the update is linear and gamma is small, the composed 100-step
operator is a banded convolution with numerically tiny bandwidth D (~12 for
gamma=0.01):
    x_100[t] = sum_{|d|<=D} G_d x0[t+d] + V_d y[t+d]
So the whole problem collapses to ONE banded-matmul pass on device:
  - time axis folded 16-way into the partition dim (16 blocks x 8 rows = 128)
  - the stencil taps become 128x128 block-banded weight matrices; taps that
    cross a fold boundary land in neighbor-column streams (sigma = -S..S)
  - per 512-col tile: (2S+1) x-matmuls + (2S+1) y-matmuls accumulate in PSUM
T is sharded across 8 cores; the first/last 128 columns (edge-rule
influenced + window zero-padding) are computed host-side on tiny strips.
"""
import os
import numpy as np

N, M, T, ITERS = 8, 4, 500000, 100
NCORES = 8
L = T // NCORES          # 62500 timesteps per core
FOLD = 16                # time-fold factor -> 16 blocks x 8 rows = 128 partitions
NC = 3908                # out cols per core: 16*3908 = 62528 >= 62500
EDGE = 128               # host-computed override width at the two true edges
STRIP = 384              # width of host edge strips
TAU = 1e-10              # tap truncation threshold (relative)

_PROGRAM_CACHE = {}
USE_BF16 = bool(int(os.environ.get("KALMAN_BF16", "1")))


def _compose_taps(F, H, Q, R, gamma):
    """Banded composition of the 100 linear steps, in float64."""
    Qinv = np.linalg.inv(Q)
    Rinv = np.linalg.inv(R)
    negQinv = -Qinv
    FtQinv = F.T @ Qinv
    HtRinv = H.T @ Rinv
    Z1 = np.eye(N); Z1[0, 0] = 0.0
    Z2 = np.eye(N); Z2[-1, -1] = 0.0
    Ap = np.eye(N) + gamma * (negQinv @ Z1 - FtQinv @ Z2 @ F - HtRinv @ H)
    Bp = -gamma * (negQinv @ Z1 @ F)
    Cp = gamma * (FtQinv @ Z2)
    Gam = gamma * HtRinv

    K = ITERS
    G = np.zeros((2 * K + 1, N, N))
    V = np.zeros((2 * K + 1, N, M))
    G[K] = np.eye(N)
    for _ in range(K):
        Gn = np.einsum("ij,djk->dik", Ap, G)
        Gn[:-1] += np.einsum("ij,djk->dik", Bp, G[1:])
        Gn[1:] += np.einsum("ij,djk->dik", Cp, G[:-1])
        Vn = np.einsum("ij,djk->dik", Ap, V)
        Vn[:-1] += np.einsum("ij,djk->dik", Bp, V[1:])
        Vn[1:] += np.einsum("ij,djk->dik", Cp, V[:-1])
        Vn[K] += Gam
        G, V = Gn, Vn

    gmax = np.abs(G).max(axis=(1, 2))
    vmax = np.abs(V).max(axis=(1, 2))
    scale = max(gmax.max(), vmax.max())
    keep = np.where((gmax > TAU * scale) | (vmax > TAU * scale))[0]
    D = int(max(1, np.abs(keep - K).max()))
    return G, V, D, (Ap.astype(np.float32), Bp.astype(np.float32),
                     Cp.astype(np.float32), Gam.astype(np.float32))


def _build_program(S):
    """Build + schedule the Bass/Tile program (cached per S)."""
    import concourse.bass as bass
    import concourse.tile as tile
    from concourse import bacc, mybir

    if S in _PROGRAM_CACHE:
        return _PROGRAM_CACHE[S]

    CW = NC + 2 * S
    nsig = 2 * S + 1
    f32 = mybir.dt.float32
    f32r = mybir.dt.bfloat16 if USE_BF16 else mybir.dt.float32r

    nc = bacc.Bacc("TRN2", target_bir_lowering=False, debug=False,
                   enable_asserts=False, num_devices=NCORES)
    xf = nc.dram_tensor("xf", [128, CW], f32r, kind="ExternalInput").ap()
    yf = nc.dram_tensor("yf", [64, CW], f32r, kind="ExternalInput").ap()
    wx = nc.dram_tensor("wx", [128, nsig * 128], f32r, kind="ExternalInput").ap()
    wy = nc.dram_tensor("wy", [64, nsig * 128], f32r, kind="ExternalInput").ap()
    out = nc.dram_tensor("out", [128, NC], f32, kind="ExternalOutput").ap()

    TS = 512
    tiles = []
    c = 0
    while c < NC:
        tiles.append((c, min(TS, NC - c)))
        c += TS

    with tile.TileContext(nc) as tc:
        with tc.tile_pool(name="consts", bufs=1) as consts, \
             tc.tile_pool(name="ps", bufs=8, space="PSUM") as ps_pool, \
             tc.tile_pool(name="outp", bufs=8) as outp:
            wxsb = consts.tile([128, nsig * 128], f32r)
            wysb = consts.tile([64, nsig * 128], f32r)
            # scalar (Activation HWDGE) carries weights + y; sync carries x.
            # The two direct-DMA streams run in parallel, so the first
            # accumulation group's operands land ~2us in instead of ~13us.
            nc.scalar.dma_start(wysb[:], wy[:])
            nc.scalar.dma_start(wxsb[:], wx[:])
            xsb = consts.tile([128, CW], f32r)
            ysb = consts.tile([64, CW], f32r)
            # chunked loads so compute can start before the whole window lands
            for (c0, cn) in tiles:
                nc.sync.dma_start(xsb[:, c0:c0 + cn], xf[:, c0:c0 + cn])
                nc.scalar.dma_start(ysb[:, c0:c0 + cn], yf[:, c0:c0 + cn])
            nc.sync.dma_start(xsb[:, NC:CW], xf[:, NC:CW])
            nc.scalar.dma_start(ysb[:, NC:CW], yf[:, NC:CW])
            for ti, (c0, cn) in enumerate(tiles):
                ps = ps_pool.tile([128, cn], f32)
                for si in range(nsig):
                    nc.tensor.matmul(
                        ps[:],
                        wysb[:, si * 128:(si + 1) * 128],
                        ysb[:, c0 + si:c0 + si + cn],
                        start=(si == 0), stop=False)
                for si in range(nsig):
                    nc.tensor.matmul(
                        ps[:],
                        wxsb[:, si * 128:(si + 1) * 128],
                        xsb[:, c0 + si:c0 + si + cn],
                        start=False, stop=(si == nsig - 1))
                ot = outp.tile([128, cn], f32)
                nc.vector.tensor_copy(ot[:], ps[:])
                eng = nc.scalar if ti % 2 else nc.sync
                eng.dma_start(out[:, c0:c0 + cn], ot[:])
    nc.compile()
    _PROGRAM_CACHE[S] = nc
    return nc


def _fold(a, rows, CW):
    # a: (rows, 16*CW) -> (rows*16 partitions, CW); partition b*rows+r holds
    # times t = c*16+b
    return np.ascontiguousarray(
        a.reshape(rows, CW, FOLD).transpose(2, 0, 1).reshape(FOLD * rows, CW))


def _run_edge_strip(x0, y, Ap, Bp, Cp, Gam):
    # reference-style edge replication on both strip ends; only the true-edge
    # side of the strip is consumed, the other side's garbage stays >100 cols
    # away from the EDGE-wide region we keep.
    x = x0.copy()
    for _ in range(ITERS):
        xp = np.concatenate([x[:, :1], x[:, :-1]], axis=1)
        xf_ = np.concatenate([x[:, 1:], x[:, -1:]], axis=1)
        x = (Ap @ x + Bp @ xp + Cp @ xf_ + Gam @ y).astype(np.float32)
    return x


def kernel(xs, ys, F, H, Q, R, gamma):
    from concourse.bass_utils import run_bass_kernel_spmd

    xs = np.asarray(xs, dtype=np.float32)
    ysv = np.asarray(ys, dtype=np.float32)
    F64 = np.asarray(F, dtype=np.float64)
    H64 = np.asarray(H, dtype=np.float64)
    Q64 = np.asarray(Q, dtype=np.float64)
    R64 = np.asarray(R, dtype=np.float64)
    g = float(np.asarray(gamma))

    G, V, D, mats32 = _compose_taps(F64, H64, Q64, R64, g)
    S = (D + FOLD - 1) // FOLD
    assert S <= 7, f"bandwidth D={D} too large for single-pass kernel"
    CW = NC + 2 * S
    nsig = 2 * S + 1

    # ---- weights ----
    K = ITERS
    WX = np.zeros((nsig, 128, 128), dtype=np.float32)
    WY = np.zeros((nsig, 64, 128), dtype=np.float32)
    for si in range(nsig):
        sig = si - S
        for bo in range(FOLD):
            for bi in range(FOLD):
                d = sig * FOLD + bi - bo
                if abs(d) > D:
                    continue
                WX[si, bi * 8:bi * 8 + 8, bo * 8:bo * 8 + 8] = G[K + d].T
                WY[si, bi * 4:bi * 4 + 4, bo * 8:bo * 8 + 8] = V[K + d].T

    # ---- per-core folded input windows ----
    pad = FOLD * S
    padR = pad + (FOLD * NC - L)          # right overhang of core 7's window
    xs_p = np.zeros((N, T + pad + padR), dtype=np.float32)
    ys_p = np.zeros((M, T + pad + padR), dtype=np.float32)
    xs_p[:, pad:pad + T] = xs
    ys_p[:, pad:pad + T] = ysv
    # SBUF weight tile is (parts, nsig*128), sigma-major along columns
    wx_np = np.ascontiguousarray(WX.transpose(1, 0, 2).reshape(128, nsig * 128))
    wy_np = np.ascontiguousarray(WY.transpose(1, 0, 2).reshape(64, nsig * 128))
    in_maps = []
    for i in range(NCORES):
        o = i * L
        in_maps.append({
            "xf": _fold(xs_p[:, o:o + FOLD * CW], N, CW),
            "yf": _fold(ys_p[:, o:o + FOLD * CW], M, CW),
            "wx": wx_np,
            "wy": wy_np,
        })

    if USE_BF16:
        import ml_dtypes
        bf16 = np.dtype(ml_dtypes.bfloat16)
        for m_ in in_maps:
            for k in m_:
                m_[k] = m_[k].astype(bf16)
    nc = _build_program(S)
    trace = bool(int(os.environ.get("KALMAN_TRACE", "0")))
    res = run_bass_kernel_spmd(nc, in_maps, core_ids=list(range(NCORES)),
                               trace=trace)
    if trace and res.exec_time_ns is not None:
        print(f"HW exec time: {res.exec_time_ns} ns")
        print(f"HW exec time mean: {res.mean_exec_time_ns} ns")

    out_full = np.empty((N, T), dtype=np.float32)
    for i in range(NCORES):
        o = i * L
        Out = res.results[i]["out"]                       # (128, NC)
        unf = Out.reshape(FOLD, N, NC).transpose(1, 2, 0).reshape(N, FOLD * NC)
        out_full[:, o:o + L] = unf[:, :L]

    # ---- host edge strips (exact edge-replication dynamics) ----
    Ap32, Bp32, Cp32, Gam32 = mats32
    left = _run_edge_strip(xs[:, :STRIP], ysv[:, :STRIP],
                           Ap32, Bp32, Cp32, Gam32)
    right = _run_edge_strip(xs[:, -STRIP:], ysv[:, -STRIP:],
                            Ap32, Bp32, Cp32, Gam32)
    out_full[:, :EDGE] = left[:, :EDGE]
    out_full[:, -EDGE:] = right[:, -EDGE:]
    return out_full
